# revision 40
# baseline (speedup 1.0000x reference)
"""ConvDeepSet kernel for Trainium2 (8 NeuronCores, Bass/Tile).

Math (per batch b, target point o, channel c):
    agg[o,c] = sum_i yd[i,c] * exp(-alpha_c * (x_i - t_o)^2)      yd = [1 | y]
    out[o,:] = [agg0, agg1/(agg0+eps), ...] @ W + b

All channels share one sigma here, so a single RBF matrix E[i,o] serves the
whole aggregation after folding W (and b) into the context values on host:
    U[i,j] = sum_{c>0} yd[i,c] W[c,j] + b[j]        (n_in, 16)
    den[o] = sum_i E[i,o]
    P[o,j] = sum_i E[i,o] U[i,j]   ( = conv@W' + den*b )
    out[o,:] = den*W[0,:] + P[o,:]/(den+eps)

The exponent -a(x-t)^2 = s*u - s^2/2 - u^2/2 with s = sqrt(2a)x, u = sqrt(2a)t
is a rank-8 fp16 matmul using 2-way fp16 splits (host-prepared), exact to
~1e-4 absolute.  ScalarE applies exp (PSUM -> SBUF fp16); aggregation runs in
fp16 with fp32 PSUM accumulation.

Banded evaluation: with length scale ~0.1 the RBF is negligible beyond
|x-t| ~ 0.5, so the host sorts context and target points and gives each
128-target chunk only its 512 nearest (contiguous in sorted order) context
points.  That halves both the exponent matmul and the exp work.  The device
program is fully static; the data-dependent window choice only changes what
the host writes into the input blocks.  A host-side bound on the dropped
tail mass falls back to the dense kernel when the banded window would not
be accurate enough (and for multi-sigma or fp16-overflow inputs).

Sharding: core c -> (batch c//2, sorted-target half c%2).  Per-core data all
lives in SBUF; the (n_in, n_out, C) intermediate never materializes.
"""

import numpy as np

B, N_IN, N_OUT = 4, 1024, 1024
IN_CH_RAW, OUT_CH = 7, 16
IN_CH = IN_CH_RAW + 1
N_CORES = 8
O_CORE = N_OUT // 2          # 512 target points per core
P = 128                      # partitions
KI = N_IN // P               # 8 contraction chunks (dense path)
NPAIR = KI // 2              # exp processed in chunk pairs (dense path)
KJ = O_CORE // P             # 4 output chunks
KEXP = 8                     # rows of the split-product exponent matmul
W_WIN = 512                  # banded context window per output chunk
KIW = W_WIN // P             # 4 window chunks
BBLK = W_WIN + P             # per-kj xr block: [lxh | rth]
EPS = 1e-8

_BASS_CACHE: dict = {}


CW = 1 + 2 * OUT_CH   # ydt columns: [density | y@W'+b | W0 broadcast]


def _build_banded():
    """Banded fp16 pipeline (single sigma group), raw Bass.

    Per kj (4 output chunks of 128 targets):
      - 4 exponent matmuls (rank 8, fp16) fill PSUM bank A[kj] [128, 512]
      - two ScalarE Exp halves produce E[kj] [128, 512] fp16 in SBUF
      - 4 aggregation matmuls accumulate pacc[kj] [128, 33] (fp32 PSUM):
        col 0 = den + eps (eps via a sacrificial window row whose
        exponent is tuned so E*ydt = 1e-8 with fp16-normal factors),
        cols 1:17 = P (+ den*b via the host b-fold),
        cols 17:33 = den*W0 (W0 broadcast into the ydt columns)
      - epilogue per kj: recip = 1/(den+eps) on DVE, prod = P*recip on
        DVE (kj0-1) or ACT Copy-with-scale (kj2-3, ACT is free after its
        Exps), out = prod + den*W0 on DVE
    Dummy matmuls warm the PE p-state while the inputs are in flight,
    then all 16 exponent matmuls run back-to-back, then the aggregations.
    One contiguous [128, 64] output DMA at the end.
    """
    import concourse.bass as bass
    from concourse import mybir

    f32 = mybir.dt.float32
    f16 = mybir.dt.float16
    Exp = mybir.ActivationFunctionType.Exp

    nc = bass.Bass("TRN2", target_bir_lowering=False, debug=False)

    xr_d = nc.dram_tensor("xr", [KEXP, KJ * BBLK], f16, kind="ExternalInput")
    ydt_d = nc.dram_tensor("ydt", [P, KJ * KIW * CW], f16,
                           kind="ExternalInput")
    out_d = nc.dram_tensor("out", [P, KJ * OUT_CH], f32, kind="ExternalOutput")

    xr_t = nc.alloc_sbuf_tensor("xr_sb", [KEXP, KJ * BBLK], f16)
    xr = xr_t.ap()
    lxh = [xr[:, kj * BBLK:kj * BBLK + W_WIN] for kj in range(KJ)]
    rth = [xr[:, kj * BBLK + W_WIN:(kj + 1) * BBLK] for kj in range(KJ)]
    ydt = nc.alloc_sbuf_tensor("ydt_sb", [P, KJ * KIW * CW], f16).ap()
    warm = nc.alloc_sbuf_tensor("warm_sb", [1, 1], f32).ap()
    dum = nc.alloc_sbuf_tensor("dum_sb", [KEXP, W_WIN], f16).ap()
    # E buffers paired per Exp wave so one activation covers two banks
    E_w = [nc.alloc_sbuf_tensor(f"e_sb{w}", [P, 2 * W_WIN], f16).ap()
           for w in range(2)]
    E = [E_w[kj // 2][:, (kj % 2) * W_WIN:(kj % 2 + 1) * W_WIN]
         for kj in range(KJ)]
    recip4 = nc.alloc_sbuf_tensor("recip4_sb", [P, KJ], f32).ap()
    prod_t = nc.alloc_sbuf_tensor("prod_sb", [P, KJ * OUT_CH], f32)
    prod = prod_t.ap()
    o_t = nc.alloc_sbuf_tensor("o_all_sb", [P, KJ * OUT_CH], f32)
    o_all = o_t.ap()
    # one A tensor spanning 4 PSUM banks; exponent matmuls are single-
    # instruction groups so sharing banks is safe (unlike pacc, where
    # multiple accumulation groups in one bank wedge the PE)
    A_all = nc.alloc_psum_tensor("a_ps", [P, KJ * W_WIN], f32).ap()
    A = [A_all[:, kj * W_WIN:(kj + 1) * W_WIN] for kj in range(KJ)]
    pacc = [nc.alloc_psum_tensor(f"pacc{kj}", [P, CW], f32).ap()
            for kj in range(KJ)]

    with (
        nc.Block(no_gpsimd_drain=True) as block,
        nc.semaphore("dsem_x") as dsem_x,   # xr kj0-1 blocks DMA
        nc.semaphore("dsem_x2") as dsem_x2, # xr kj2-3 blocks DMA
        nc.semaphore("dsem_y") as dsem_y,   # ydt input DMA
        nc.semaphore("wsem") as wsem,       # warmup dummy operand memset
        nc.semaphore("psem") as psem,       # PE exponent groups done (per kj)
        nc.semaphore("asem") as asem,       # ACT exp done (per kj)
        nc.semaphore("gsem") as gsem,       # PE agg group done (per kj)
        nc.semaphore("rsem") as rsem,       # DVE recip written (per kj)
        nc.semaphore("qsem") as qsem,       # DVE prod written (kj0-1)
        nc.semaphore("tsem") as tsem,       # ACT prod written (kj2-3)
        nc.semaphore("vsem") as vsem,       # DVE final add done
        nc.semaphore("osem") as osem,       # out DMA done (unwaited)
    ):
        @block.sync
        def _(sync):
            # split xr: kj0-1 blocks complete (and their semaphore fires)
            # ahead of kj2-3, so the first exponent matmuls start sooner
            sync.dma_start(out=xr[:, 0:2 * BBLK],
                           in_=xr_d[:, 0:2 * BBLK]).then_inc(dsem_x, 16)
            sync.dma_start(out=xr[:, 2 * BBLK:],
                           in_=xr_d[:, 2 * BBLK:]).then_inc(dsem_x2, 16)
            sync.wait_ge(vsem, KJ)
            sync.dma_start(out=out_d[:], in_=o_all[:]).then_inc(osem, 16)
            # no completion wait: the Block-exit DRAIN on SP already blocks
            # until the DGE queues are empty

        @block.gpsimd
        def _(gpsimd):
            gpsimd.memset(dum[:], 0.0).then_inc(wsem, 1)

        @block.tensor
        def _(tensor):
            # dummy matmuls ramp the PE p-state toward full clock while the
            # input DMAs are still in flight (A[0] is reset by the first
            # real matmul's start=True)
            tensor.wait_ge(wsem, 1)
            for _i in range(3):
                tensor.matmul(A[0][:], dum[:, 0:P], dum[:],
                              start=True, stop=True, skip_group_check=True)
            for _i in range(9):
                tensor.matmul(A[0][:, 0:P], dum[:, 0:P], dum[:, 0:P],
                              start=True, stop=True, skip_group_check=True)
            tensor.wait_ge(dsem_x, 16)   # xr kj0-1 blocks resident
            # all 16 exponent matmuls back-to-back (PE stays busy/ramped)
            for kj in range(KJ):
                if kj == 2:
                    tensor.wait_ge(dsem_x2, 16)   # remaining xr resident
                for ki in range(KIW):
                    mm = tensor.matmul(
                        A[kj][:, ki * P:(ki + 1) * P],
                        lxh[kj][:, ki * P:(ki + 1) * P],
                        rth[kj][:],
                        start=True,
                        stop=True,
                        skip_group_check=True,
                    )
                    if ki == KIW - 1:
                        mm.then_inc(psem, 1)
            tensor.wait_ge(dsem_y, 16)   # ydt resident
            for kj in range(KJ):
                tensor.wait_ge(asem, kj + 1)
                for ki in range(KIW):
                    mm = tensor.matmul(
                        pacc[kj][:, 0:CW],
                        E[kj][:, ki * P:(ki + 1) * P],
                        ydt[:, (kj * KIW + ki) * CW:(kj * KIW + ki + 1) * CW],
                        start=(ki == 0),
                        stop=(ki == KIW - 1),
                        skip_group_check=True,
                    )
                    if ki == KIW - 1:
                        mm.then_inc(gsem, 1)

        @block.scalar
        def _(scalar):
            # ydt DMA rides the scalar queue (sync is busy with xr);
            # issued before the warm-up so the transfer overlaps table load
            scalar.dma_start(out=ydt[:], in_=ydt_d[:]).then_inc(dsem_y, 16)
            # touch Exp before the pipeline needs it: loads the ACT table
            # while the input DMAs are still in flight
            scalar.activation(warm[:], nc.const_aps.tensor(0.0, (1, 1)), Exp)
            for kj in range(KJ):
                scalar.wait_ge(psem, kj + 1)
                scalar.activation(E[kj][:], A[kj][:], Exp).then_inc(asem, 1)
            # after the Exps, ACT picks up kj2's P*recip so the DVE epilogue
            # isn't serialized on one engine (kj3's stays on DVE: routing it
            # through ACT would put two sem hops on the final critical add)
            Copy = mybir.ActivationFunctionType.Copy
            scalar.wait_ge(rsem, 3)
            scalar.activation(
                prod[:, 2 * OUT_CH:3 * OUT_CH],
                pacc[2][:, 1:1 + OUT_CH],
                Copy,
                scale=recip4[:, 2:3],
            ).then_inc(tsem, 1)

        @block.vector
        def _(vector):
            # per-kj: recip = 1/(den+eps), prod = P*recip (PSUM->SBUF),
            # pipelined so the self-pipe waits are met when reached; then
            # one strided 3D add folds in the den*W0 columns for all kj
            def recip_op(kj):
                vector.wait_ge(gsem, kj + 1)
                vector.reciprocal(
                    recip4[:, kj:kj + 1], pacc[kj][:, 0:1]
                ).then_inc(rsem, 1)

            def mul_op(kj):
                vector.wait_ge(rsem, kj + 1)
                vector.tensor_scalar_mul(
                    prod[:, kj * OUT_CH:(kj + 1) * OUT_CH],
                    pacc[kj][:, 1:1 + OUT_CH],
                    recip4[:, kj:kj + 1],
                ).then_inc(qsem, 1)

            def add_op(kj, sem, val):
                vector.wait_ge(sem, val)
                vector.tensor_tensor(
                    o_all[:, kj * OUT_CH:(kj + 1) * OUT_CH],
                    pacc[kj][:, 1 + OUT_CH:CW],
                    prod[:, kj * OUT_CH:(kj + 1) * OUT_CH],
                    op=mybir.AluOpType.add,
                ).then_inc(vsem, 1)

            # DVE handles kj0/kj1/kj3's P*recip; ACT (free after its Exps)
            # handles kj2's, so the tails run in parallel and kj3's final
            # add never crosses engines
            recip_op(0); mul_op(0)
            recip_op(1); mul_op(1)
            add_op(0, qsem, 1)
            recip_op(2)
            add_op(1, qsem, 2)
            recip_op(3); mul_op(3)
            add_op(2, tsem, 1)
            add_op(3, qsem, 3)

    return nc


def _build_fp16_raw(widths):
    """Dense fallback: raw-Bass fp16 pipeline over all 1024 context points
    (used when the banded window bound is not accurate enough)."""
    import concourse.bass as bass
    from concourse import mybir

    f32 = mybir.dt.float32
    f16 = mybir.dt.float16
    G = len(widths)
    wtot = sum(widths)
    offs = np.cumsum([0] + list(widths))
    npair = NPAIR * G
    Exp = mybir.ActivationFunctionType.Exp

    nc = bass.Bass("TRN2", target_bir_lowering=False, debug=False)

    # xr = per-group [lxh | rth] blocks along the free dim (groups must sit at
    # partition 0 for PE); one DMA feeds the first matmul. wbb = [wb0 | bt].
    BLK = N_IN + O_CORE
    xr_d = nc.dram_tensor("xr", [KEXP, G * BLK], f16, kind="ExternalInput")
    ydt_d = nc.dram_tensor("ydt", [P, KI * wtot], f16, kind="ExternalInput")
    wbb_d = nc.dram_tensor("wbb", [P, 2 * OUT_CH], f32, kind="ExternalInput")
    out_d = nc.dram_tensor("out", [O_CORE, OUT_CH], f32, kind="ExternalOutput")

    xr = nc.alloc_sbuf_tensor("xr_sb", [KEXP, G * BLK], f16).ap()
    lxh = [xr[:, g * BLK:g * BLK + N_IN] for g in range(G)]
    rth = [xr[:, g * BLK + N_IN:(g + 1) * BLK] for g in range(G)]
    ydt = nc.alloc_sbuf_tensor("ydt_sb", [P, KI * wtot], f16).ap()
    wbb = nc.alloc_sbuf_tensor("wbb_sb", [P, 2 * OUT_CH], f32).ap()
    wb0 = wbb[:, :OUT_CH]
    bt = wbb[:, OUT_CH:]
    warm = nc.alloc_sbuf_tensor("warm_sb", [1, 1], f32).ap()
    E = [nc.alloc_sbuf_tensor(f"e_sb{q}", [P, 2 * O_CORE], f16).ap()
         for q in range(npair)]
    denp = [nc.alloc_sbuf_tensor(f"denp_sb{kj}", [P, 1], f32).ap()
            for kj in range(KJ)]
    recip = [nc.alloc_sbuf_tensor(f"recip_sb{kj}", [P, 1], f32).ap()
             for kj in range(KJ)]
    t1 = [nc.alloc_sbuf_tensor(f"t1_sb{kj}", [P, OUT_CH], f32).ap()
          for kj in range(KJ)]
    o_all = nc.alloc_sbuf_tensor("o_all_sb", [P, KJ * OUT_CH], f32).ap()
    o_sb = [o_all[:, kj * OUT_CH:(kj + 1) * OUT_CH] for kj in range(KJ)]
    A = [nc.alloc_psum_tensor(f"a_ps{i}", [P, 2 * O_CORE], f32).ap()
         for i in range(2)]
    pacc = [nc.alloc_psum_tensor(f"pacc{kj}", [P, 1 + OUT_CH], f32).ap()
            for kj in range(KJ)]

    with (
        nc.Block() as block,
        nc.semaphore("dsem_s") as dsem_s,   # sync-queue input DMAs
        nc.semaphore("dsem_g") as dsem_g,   # gpsimd-queue input DMAs
        nc.semaphore("psem") as psem,       # PE exponent matmuls done
        nc.semaphore("asem") as asem,       # ACT exp pairs done
        nc.semaphore("gsem") as gsem,       # PE agg per-kj done
        nc.semaphore("vsem") as vsem,       # DVE epilogue per-kj done
        nc.semaphore("vv") as vv,           # DVE same-engine pipeline sync
        nc.semaphore("osem") as osem,       # out DMAs done
    ):
        @block.sync
        def _(sync):
            sync.dma_start(out=xr[:], in_=xr_d[:]).then_inc(dsem_s, 16)
            for kj in range(KJ):
                sync.wait_ge(vsem, kj + 1)
                sync.dma_start(
                    out=out_d[kj * P:(kj + 1) * P, :], in_=o_sb[kj][:]
                ).then_inc(osem, 16)

        @block.gpsimd
        def _(gpsimd):
            gpsimd.dma_start(out=ydt[:], in_=ydt_d[:]).then_inc(dsem_g, 16)
            gpsimd.dma_start(out=wbb[:], in_=wbb_d[:]).then_inc(dsem_g, 16)

        @block.tensor
        def _(tensor):
            tensor.wait_ge(dsem_s, 16)
            for q in range(npair):
                g, p = divmod(q, NPAIR)
                if q >= 2:
                    tensor.wait_ge(asem, q - 1)  # A buffer q%2 free again
                for half in range(2):
                    ki = 2 * p + half
                    tensor.matmul(
                        A[q % 2][:, half * O_CORE:(half + 1) * O_CORE],
                        lxh[g][:, ki * P:(ki + 1) * P],
                        rth[g][:],
                        start=True,
                        stop=True,
                    ).then_inc(psem, 1)
            tensor.wait_ge(dsem_g, 32)  # ydt (all gpsimd-queue DMAs)
            n_mm = G * KI
            for kj in range(KJ):
                for g in range(G):
                    w = widths[g]
                    coff = 0 if g == 0 else 1
                    for ki in range(KI):
                        idx = g * KI + ki
                        q = g * NPAIR + ki // 2
                        if kj == 0 and ki % 2 == 0:
                            tensor.wait_ge(asem, q + 1)
                        rhs = ydt[:, KI * offs[g] + ki * w:
                                  KI * offs[g] + (ki + 1) * w]
                        lhs = E[q][:, (ki % 2) * O_CORE + kj * P:
                                   (ki % 2) * O_CORE + (kj + 1) * P]
                        mm = tensor.matmul(
                            pacc[kj][:, coff:coff + w],
                            lhs,
                            rhs,
                            start=(idx == 0),
                            stop=(idx == n_mm - 1),
                            skip_group_check=(G > 1),
                        )
                        if idx == n_mm - 1:
                            mm.then_inc(gsem, 1)

        @block.scalar
        def _(scalar):
            scalar.activation(warm[:], nc.const_aps.tensor(0.0, (1, 1)), Exp)
            for q in range(npair):
                scalar.wait_ge(psem, 2 * (q + 1))
                scalar.activation(E[q][:], A[q % 2][:], Exp).then_inc(asem, 1)

        @block.vector
        def _(vector):
            vector.wait_ge(dsem_g, 32)  # wbb resident
            for kj in range(KJ):
                vector.wait_ge(gsem, kj + 1)
                vector.tensor_scalar_add(
                    denp[kj][:], pacc[kj][:, 0:1], EPS
                ).then_inc(vv, 1)
                vector.wait_ge(vv, 3 * kj + 1)  # denp through the pipe
                vector.scalar_tensor_tensor(
                    t1[kj][:], wb0[:], denp[kj][:], bt[:],
                    op0=mybir.AluOpType.mult, op1=mybir.AluOpType.add,
                ).then_inc(vv, 1)
                vector.reciprocal(recip[kj][:], denp[kj][:]).then_inc(vv, 1)
                vector.wait_ge(vv, 3 * kj + 3)  # t1 + recip through the pipe
                vector.scalar_tensor_tensor(
                    o_sb[kj][:], pacc[kj][:, 1:1 + OUT_CH], recip[kj][:], t1[kj][:],
                    op0=mybir.AluOpType.mult, op1=mybir.AluOpType.add,
                ).then_inc(vsem, 1)

    return nc


def _build_fp32(widths):
    """Fallback: fp32 rank-3 exponent matmul + fp32 aggregation (slower,
    used only when fp16 split values would overflow)."""
    import concourse.bacc as bacc
    import concourse.tile as tile
    from concourse import mybir

    f32 = mybir.dt.float32
    G = len(widths)
    wtot = sum(widths)
    offs = np.cumsum([0] + list(widths))

    nc = bacc.Bacc("TRN2", target_bir_lowering=False, debug=False)

    lx_d = nc.dram_tensor("lx", [3, N_IN], f32, kind="ExternalInput")
    rt_d = nc.dram_tensor("rt", [3 * G, O_CORE], f32, kind="ExternalInput")
    ydt_d = nc.dram_tensor("ydt", [P, KI * wtot], f32, kind="ExternalInput")
    wb0_d = nc.dram_tensor("wb0", [P, OUT_CH], f32, kind="ExternalInput")
    bt_d = nc.dram_tensor("bt", [P, OUT_CH], f32, kind="ExternalInput")
    out_d = nc.dram_tensor("out", [O_CORE, OUT_CH], f32, kind="ExternalOutput")

    with tile.TileContext(nc) as tc:
        with (
            tc.tile_pool(name="const", bufs=1) as cpool,
            tc.tile_pool(name="epool", bufs=1) as epool,
            tc.tile_pool(name="small", bufs=2) as spool,
            tc.tile_pool(name="outp", bufs=2) as opool,
            tc.tile_pool(name="apsum", bufs=3, space="PSUM") as apsum,
            tc.tile_pool(name="ppsum", bufs=1, space="PSUM") as ppsum,
        ):
            lx = cpool.tile([3, N_IN], f32, tag="lx")
            nc.sync.dma_start(lx[:], lx_d[:])
            rt = cpool.tile([3 * G, O_CORE], f32, tag="rt")
            nc.scalar.dma_start(rt[:], rt_d[:])
            ydt = cpool.tile([P, KI * wtot], f32, tag="ydt")
            nc.gpsimd.dma_start(ydt[:], ydt_d[:])
            wb0 = cpool.tile([P, OUT_CH], f32, tag="wb0")
            nc.gpsimd.dma_start(wb0[:], wb0_d[:])
            bt = cpool.tile([P, OUT_CH], f32, tag="bt")
            nc.gpsimd.dma_start(bt[:], bt_d[:])

            E = {}
            for g in range(G):
                for ki in range(KI):
                    a_ps = apsum.tile([P, O_CORE], f32, tag="A", name="a_ps")
                    nc.tensor.matmul(
                        a_ps[:],
                        lx[:, ki * P:(ki + 1) * P],
                        rt[3 * g:3 * g + 3, :],
                        start=True,
                        stop=True,
                    )
                    e = epool.tile([P, O_CORE], f32, tag=f"E{g}_{ki}", name="e")
                    nc.scalar.activation(
                        e[:], a_ps[:], mybir.ActivationFunctionType.Exp
                    )
                    E[(g, ki)] = e

            pacc = [
                ppsum.tile([P, 1 + OUT_CH], f32, tag=f"P{kj}", name=f"pacc{kj}")
                for kj in range(KJ)
            ]
            n_mm = G * KI
            for kj in range(KJ):
                for g in range(G):
                    w = widths[g]
                    coff = 0 if g == 0 else 1
                    for ki in range(KI):
                        idx = g * KI + ki
                        rhs = ydt[:, KI * offs[g] + ki * w: KI * offs[g] + (ki + 1) * w]
                        nc.tensor.matmul(
                            pacc[kj][:, coff:coff + w],
                            E[(g, ki)][:, kj * P:(kj + 1) * P],
                            rhs,
                            start=(idx == 0),
                            stop=(idx == n_mm - 1),
                            skip_group_check=(G > 1),
                        )

                denp = spool.tile([P, 1], f32, tag="denp", name="denp")
                nc.vector.tensor_scalar_add(denp[:], pacc[kj][:, 0:1], EPS)
                recip = spool.tile([P, 1], f32, tag="recip", name="recip")
                nc.vector.reciprocal(recip[:], denp[:])
                t1 = spool.tile([P, OUT_CH], f32, tag="t1", name="t1")
                nc.vector.scalar_tensor_tensor(
                    t1[:], wb0[:], denp[:], bt[:],
                    op0=mybir.AluOpType.mult, op1=mybir.AluOpType.add,
                )
                o_sb = opool.tile([P, OUT_CH], f32, tag="osb", name="o_sb")
                nc.vector.scalar_tensor_tensor(
                    o_sb[:], pacc[kj][:, 1:1 + OUT_CH], recip[:], t1[:],
                    op0=mybir.AluOpType.mult, op1=mybir.AluOpType.add,
                )
                nc.sync.dma_start(out_d[kj * P:(kj + 1) * P, :], o_sb[:])

    nc.compile()
    return nc


def _split2_f16(v):
    """2-way fp16 split: v ~= h1 + h2 with each half exactly fp16."""
    v = v.astype(np.float32)
    h1 = v.astype(np.float16)
    h2 = (v - h1.astype(np.float32)).astype(np.float16)
    return h1, h2


def _prepare_banded(context_x, context_y, t, sigma, W, b):
    """Host prep for the banded kernel. Returns (in_maps, gathers) or None
    when the banded path does not apply (multi-sigma, fp16 overflow risk,
    or a window accuracy bound too loose)."""
    sigma = np.asarray(sigma, dtype=np.float32)
    if not np.all(sigma == sigma[0]):
        return None
    a = 0.5 / np.exp(2.0 * np.float64(sigma[0]))
    if not np.isfinite(a):
        return None
    r = np.sqrt(2.0 * a)

    cx = np.asarray(context_x, dtype=np.float64)[:, :, 0]
    tt = np.asarray(t, dtype=np.float64)[:, :, 0]
    xmax = max(float(np.abs(cx).max()), float(np.abs(tt).max()), 1.0)
    if not (a * xmax * xmax < 3e4):
        return None

    W64 = np.asarray(W, dtype=np.float64)
    b64 = np.asarray(b, dtype=np.float64)
    WR = W_WIN - 1   # real context points per window; slot 511 is the eps row

    in_maps, gathers = [], []
    for core in range(N_CORES):
        bidx, half = core // 2, core % 2
        xo = np.argsort(cx[bidx], kind="stable")
        xs = cx[bidx][xo]
        to = np.argsort(tt[bidx], kind="stable")
        ts = tt[bidx][to]
        # folded context values in sorted-x order: [1 | y@W' + b | W0]
        u = np.asarray(context_y[bidx], np.float64) @ W64[1:] + b64
        u_s = u[xo]

        xr = np.empty((KEXP, KJ * BBLK), dtype=np.float16)
        ydt = np.empty((P, KJ * KIW * CW), dtype=np.float16)
        tidx = np.empty(O_CORE, dtype=np.int64)
        cand = np.arange(0, N_IN - WR + 1)
        for kj in range(KJ):
            lo = half * O_CORE + kj * P
            tc = ts[lo:lo + P]
            tidx[kj * P:(kj + 1) * P] = to[lo:lo + P]
            # window start maximizing the smaller margin
            m = np.minimum(tc.min() - xs[cand], xs[cand + WR - 1] - tc.max())
            s = int(cand[np.argmax(m)])
            xw = xs[s:s + WR]
            # accuracy bound: dropped tail mass on the density channel
            drop = np.concatenate([xs[:s], xs[s + WR:]])
            if drop.size:
                d = np.maximum(0.0, np.maximum(tc.min() - drop,
                                               drop - tc.max()))
                if float(np.exp(-a * d * d).sum()) > 5e-4:
                    return None
            s1, s2 = _split2_f16(r * xw)
            u1, u2 = _split2_f16(r * tc)
            q1, q2 = _split2_f16(0.5 * (r * xw) ** 2)
            w1, w2 = _split2_f16(0.5 * (r * tc) ** 2)
            one_i = np.ones(WR, np.float16)
            neg1 = np.full(P, -1.0, np.float16)
            lblk = np.zeros((KEXP, W_WIN), dtype=np.float16)
            lblk[:, :WR] = np.stack([s1, s1, s2, s2, q1, q2, one_i, one_i])
            # eps row (slot 511): exponent = -9.21034 -> E = 1e-4, and the
            # density ydt value is 1e-4, so E*ydt = 1e-8 = eps with both
            # factors fp16-normal (1e-8 itself would underflow fp16)
            lblk[4, WR] = np.float16(9.21034)
            xr[:, kj * BBLK:kj * BBLK + W_WIN] = lblk
            xr[:, kj * BBLK + W_WIN:(kj + 1) * BBLK] = np.stack(
                [u1, u2, u1, u2, neg1, neg1, -w1, -w2]
            )
            blk = np.zeros((W_WIN, CW))
            blk[:WR, 0] = 1.0
            blk[:WR, 1:1 + OUT_CH] = u_s[s:s + WR]
            blk[:WR, 1 + OUT_CH:] = W64[0][None, :]
            blk[WR, 0] = 1e-4   # with E = 1e-4: pacc[:,0] = den + 1e-8
            ydt[:, kj * KIW * CW:(kj + 1) * KIW * CW] = (
                blk.reshape(KIW, P, CW).transpose(1, 0, 2).reshape(P, KIW * CW)
            ).astype(np.float16)

        in_maps.append({"xr": xr, "ydt": ydt})
        gathers.append((bidx, tidx))
    return in_maps, gathers


def _prepare_inputs(context_x, context_y, t, sigma, W, b):
    """Dense-path host prep: group channels by sigma, fold W, build
    per-core inputs."""
    sigma = np.asarray(sigma, dtype=np.float32)
    W64 = np.asarray(W, dtype=np.float64)
    b64 = np.asarray(b, dtype=np.float64)

    uniq = []
    for c in range(IN_CH):
        if sigma[c] not in uniq:
            uniq.append(sigma[c])
    uniq.sort(key=lambda s: (s != sigma[0]))  # channel-0 group first
    groups = [[c for c in range(IN_CH) if sigma[c] == s] for s in uniq]
    alphas = [0.5 / np.exp(2.0 * np.float64(s)) for s in uniq]
    widths = tuple((1 + OUT_CH) if 0 in g else OUT_CH for g in groups)
    G = len(groups)

    xmax = max(
        float(np.abs(np.asarray(context_x)).max()),
        float(np.abs(np.asarray(t)).max()),
        1.0,
    )
    fp16_ok = all(a * xmax * xmax < 3e4 and np.isfinite(a) for a in alphas)

    in_maps = []
    for core in range(N_CORES):
        bidx, half = core // 2, core % 2
        x = np.asarray(context_x[bidx, :, 0], dtype=np.float64)
        th = np.asarray(t[bidx, half * O_CORE:(half + 1) * O_CORE, 0],
                        dtype=np.float64)
        y = np.asarray(context_y[bidx], dtype=np.float64)

        m = {}
        if fp16_ok:
            BLK = N_IN + O_CORE
            xr = np.empty((KEXP, G * BLK), dtype=np.float16)
            for g, a in enumerate(alphas):
                r = np.sqrt(2.0 * a)
                s1, s2 = _split2_f16(r * x)
                u1, u2 = _split2_f16(r * th)
                q1, q2 = _split2_f16(0.5 * (r * x) ** 2)
                w1, w2 = _split2_f16(0.5 * (r * th) ** 2)
                one_i = np.ones(N_IN, np.float16)
                neg1 = np.full(O_CORE, -1.0, np.float16)
                xr[:, g * BLK:g * BLK + N_IN] = np.stack(
                    [s1, s1, s2, s2, q1, q2, one_i, one_i]
                )
                xr[:, g * BLK + N_IN:(g + 1) * BLK] = np.stack(
                    [u1, u2, u1, u2, neg1, neg1, -w1, -w2]
                )
            m["xr"] = xr
        else:
            lx = np.stack([x, x * x, np.ones_like(x)]).astype(np.float32)
            rt = np.empty((3 * G, O_CORE), dtype=np.float32)
            for g, a in enumerate(alphas):
                rt[3 * g + 0] = 2.0 * a * th
                rt[3 * g + 1] = -a
                rt[3 * g + 2] = -a * th * th
            m["lx"], m["rt"] = lx, rt

        blocks = []
        for g, chans in enumerate(groups):
            w = widths[g]
            rhs = np.zeros((N_IN, w), dtype=np.float64)
            coff = 0
            if 0 in chans:
                rhs[:, 0] = 1.0
                coff = 1
            conv_ch = [c for c in chans if c > 0]
            if conv_ch:
                rhs[:, coff:] = y[:, [c - 1 for c in conv_ch]] @ W64[conv_ch, :]
            blocks.append(
                rhs.reshape(KI, P, w).transpose(1, 0, 2).reshape(P, KI * w)
            )
        ydt = np.concatenate(blocks, axis=1)
        m["ydt"] = ydt.astype(np.float16 if fp16_ok else np.float32)
        wb0 = np.tile(W64[0].astype(np.float32), (P, 1))
        bt = np.tile(b64.astype(np.float32), (P, 1))
        if fp16_ok:
            m["wbb"] = np.concatenate([wb0, bt], axis=1)
        else:
            m["wb0"], m["bt"] = wb0, bt
        in_maps.append(m)
    return widths, fp16_ok, in_maps


def _run(inputs: dict, trace: bool = False):
    """Compile (cached), run on 8 cores, gather. Returns (output, results)."""
    from concourse.bass_utils import run_bass_kernel_spmd

    banded = _prepare_banded(
        inputs["context_x"], inputs["context_y"], inputs["t"],
        inputs["sigma"], inputs["W"], inputs["b"],
    )
    out = np.empty((B, N_OUT, OUT_CH), dtype=np.float32)
    if banded is not None:
        in_maps, gathers = banded
        if "banded" not in _BASS_CACHE:
            _BASS_CACHE["banded"] = _build_banded()
        nc = _BASS_CACHE["banded"]
        res = run_bass_kernel_spmd(nc, in_maps, list(range(N_CORES)),
                                   trace=trace)
        for core in range(N_CORES):
            bidx, tidx = gathers[core]
            r = res.results[core]["out"]  # [128, KJ*16]
            out[bidx, tidx, :] = (
                r.reshape(P, KJ, OUT_CH).transpose(1, 0, 2).reshape(O_CORE,
                                                                    OUT_CH)
            )
        return out, res

    widths, fp16_ok, in_maps = _prepare_inputs(
        inputs["context_x"], inputs["context_y"], inputs["t"],
        inputs["sigma"], inputs["W"], inputs["b"],
    )
    key = (widths, fp16_ok)
    if key not in _BASS_CACHE:
        _BASS_CACHE[key] = (_build_fp16_raw if fp16_ok else _build_fp32)(widths)
    nc = _BASS_CACHE[key]

    res = run_bass_kernel_spmd(nc, in_maps, list(range(N_CORES)), trace=trace)

    for core in range(N_CORES):
        bidx, half = core // 2, core % 2
        out[bidx, half * O_CORE:(half + 1) * O_CORE, :] = res.results[core]["out"]
    return out, res


def kernel(**inputs) -> np.ndarray:
    out, _ = _run(inputs, trace=False)
    return out


# revision 44
# speedup vs baseline: 1.0259x; 1.0259x over previous
"""ConvDeepSet kernel for Trainium2 (8 NeuronCores, Bass/Tile).

Math (per batch b, target point o, channel c):
    agg[o,c] = sum_i yd[i,c] * exp(-alpha_c * (x_i - t_o)^2)      yd = [1 | y]
    out[o,:] = [agg0, agg1/(agg0+eps), ...] @ W + b

All channels share one sigma here, so a single RBF matrix E[i,o] serves the
whole aggregation after folding W (and b) into the context values on host:
    U[i,j] = sum_{c>0} yd[i,c] W[c,j] + b[j]        (n_in, 16)
    den[o] = sum_i E[i,o]
    P[o,j] = sum_i E[i,o] U[i,j]   ( = conv@W' + den*b )
    out[o,:] = den*W[0,:] + P[o,:]/(den+eps)

The exponent -a(x-t)^2 = s*u - s^2/2 - u^2/2 with s = sqrt(2a)x, u = sqrt(2a)t
is a rank-8 fp16 matmul using 2-way fp16 splits (host-prepared), exact to
~1e-4 absolute.  ScalarE applies exp (PSUM -> SBUF fp16); aggregation runs in
fp16 with fp32 PSUM accumulation.

Banded evaluation: with length scale ~0.1 the RBF is negligible beyond
|x-t| ~ 0.5, so the host sorts context and target points and gives each
128-target chunk only its 512 nearest (contiguous in sorted order) context
points.  That halves both the exponent matmul and the exp work.  The device
program is fully static; the data-dependent window choice only changes what
the host writes into the input blocks.  A host-side bound on the dropped
tail mass falls back to the dense kernel when the banded window would not
be accurate enough (and for multi-sigma or fp16-overflow inputs).

Sharding: core c -> (batch c//2, sorted-target half c%2).  Per-core data all
lives in SBUF; the (n_in, n_out, C) intermediate never materializes.
"""

import numpy as np

B, N_IN, N_OUT = 4, 1024, 1024
IN_CH_RAW, OUT_CH = 7, 16
IN_CH = IN_CH_RAW + 1
N_CORES = 8
O_CORE = N_OUT // 2          # 512 target points per core
P = 128                      # partitions
KI = N_IN // P               # 8 contraction chunks (dense path)
NPAIR = KI // 2              # exp processed in chunk pairs (dense path)
KJ = O_CORE // P             # 4 output chunks
KEXP = 8                     # rows of the split-product exponent matmul
W_WIN = 512                  # banded context window per output chunk
KIW = W_WIN // P             # 4 window chunks
BBLK = W_WIN + P             # per-kj xr block: [lxh | rth]
EPS = 1e-8

_BASS_CACHE: dict = {}


CW = 1 + 2 * OUT_CH   # ydt columns: [density | y@W'+b | W0 broadcast]


def _build_banded():
    """Banded fp16 pipeline (single sigma group), raw Bass.

    Per kj (4 output chunks of 128 targets):
      - 4 exponent matmuls (rank 8, fp16) fill PSUM bank A[kj] [128, 512]
      - two ScalarE Exp halves produce E[kj] [128, 512] fp16 in SBUF
      - 4 aggregation matmuls accumulate pacc[kj] [128, 33] (fp32 PSUM):
        col 0 = den + eps (eps via a sacrificial window row whose
        exponent is tuned so E*ydt = 1e-8 with fp16-normal factors),
        cols 1:17 = P (+ den*b via the host b-fold),
        cols 17:33 = den*W0 (W0 broadcast into the ydt columns)
      - epilogue per kj: recip = 1/(den+eps) on DVE, prod = P*recip on
        DVE (kj0-1) or ACT Copy-with-scale (kj2-3, ACT is free after its
        Exps), out = prod + den*W0 on DVE
    Dummy matmuls warm the PE p-state while the inputs are in flight,
    then all 16 exponent matmuls run back-to-back, then the aggregations.
    One contiguous [128, 64] output DMA at the end.
    """
    import concourse.bass as bass
    from concourse import mybir

    f32 = mybir.dt.float32
    f16 = mybir.dt.float16
    Exp = mybir.ActivationFunctionType.Exp

    nc = bass.Bass("TRN2", target_bir_lowering=False, debug=False)

    xr_d = nc.dram_tensor("xr", [KEXP, KJ * BBLK], f16, kind="ExternalInput")
    ydt_d = nc.dram_tensor("ydt", [P, KJ * KIW * CW], f16,
                           kind="ExternalInput")
    out_d = nc.dram_tensor("out", [P, KJ * OUT_CH], f32, kind="ExternalOutput")

    xr_t = nc.alloc_sbuf_tensor("xr_sb", [KEXP, KJ * BBLK], f16)
    xr = xr_t.ap()
    lxh = [xr[:, kj * BBLK:kj * BBLK + W_WIN] for kj in range(KJ)]
    rth = [xr[:, kj * BBLK + W_WIN:(kj + 1) * BBLK] for kj in range(KJ)]
    ydt = nc.alloc_sbuf_tensor("ydt_sb", [P, KJ * KIW * CW], f16).ap()
    warm = nc.alloc_sbuf_tensor("warm_sb", [1, 1], f32).ap()
    dum = nc.alloc_sbuf_tensor("dum_sb", [KEXP, W_WIN], f16).ap()
    # E buffers paired per Exp wave so one activation covers two banks
    E_w = [nc.alloc_sbuf_tensor(f"e_sb{w}", [P, 2 * W_WIN], f16).ap()
           for w in range(2)]
    E = [E_w[kj // 2][:, (kj % 2) * W_WIN:(kj % 2 + 1) * W_WIN]
         for kj in range(KJ)]
    recip4 = nc.alloc_sbuf_tensor("recip4_sb", [P, KJ], f32).ap()
    prod_t = nc.alloc_sbuf_tensor("prod_sb", [P, KJ * OUT_CH], f32)
    prod = prod_t.ap()
    o_t = nc.alloc_sbuf_tensor("o_all_sb", [P, KJ * OUT_CH], f32)
    o_all = o_t.ap()
    # one A tensor spanning 4 PSUM banks; exponent matmuls are single-
    # instruction groups so sharing banks is safe (unlike pacc, where
    # multiple accumulation groups in one bank wedge the PE)
    A_all = nc.alloc_psum_tensor("a_ps", [P, KJ * W_WIN], f32).ap()
    A = [A_all[:, kj * W_WIN:(kj + 1) * W_WIN] for kj in range(KJ)]
    pacc = [nc.alloc_psum_tensor(f"pacc{kj}", [P, CW], f32).ap()
            for kj in range(KJ)]

    with (
        nc.Block(no_gpsimd_drain=True) as block,
        nc.semaphore("dsem_x") as dsem_x,   # xr kj0-1 blocks DMA
        nc.semaphore("dsem_x2") as dsem_x2, # xr kj2-3 blocks DMA
        nc.semaphore("dsem_y") as dsem_y,   # ydt input DMA
        nc.semaphore("wsem") as wsem,       # warmup dummy operand memset
        nc.semaphore("psem") as psem,       # PE exponent groups done (per kj)
        nc.semaphore("asem") as asem,       # ACT exp done (per kj)
        nc.semaphore("gsem") as gsem,       # PE agg group done (per kj)
        nc.semaphore("rsem") as rsem,       # DVE recip written (per kj)
        nc.semaphore("qsem") as qsem,       # DVE prod written (kj0-1)
        nc.semaphore("tsem") as tsem,       # ACT prod written (kj2-3)
        nc.semaphore("vsem") as vsem,       # DVE final add done
        nc.semaphore("osem") as osem,       # out DMA done (unwaited)
    ):
        @block.sync
        def _(sync):
            # split xr: kj0-1 blocks complete (and their semaphore fires)
            # ahead of kj2-3, so the first exponent matmuls start sooner
            sync.dma_start(out=xr[:, 0:2 * BBLK],
                           in_=xr_d[:, 0:2 * BBLK]).then_inc(dsem_x, 16)
            sync.dma_start(out=xr[:, 2 * BBLK:],
                           in_=xr_d[:, 2 * BBLK:]).then_inc(dsem_x2, 16)
            sync.wait_ge(vsem, KJ)
            sync.dma_start(out=out_d[:], in_=o_all[:]).then_inc(osem, 16)
            # no completion wait: the Block-exit DRAIN on SP already blocks
            # until the DGE queues are empty

        @block.gpsimd
        def _(gpsimd):
            gpsimd.memset(dum[:], 0.0).then_inc(wsem, 1)

        @block.tensor
        def _(tensor):
            # dummy matmuls ramp the PE p-state toward full clock while the
            # input DMAs are still in flight (A[0] is reset by the first
            # real matmul's start=True)
            tensor.wait_ge(wsem, 1)
            for _i in range(3):
                tensor.matmul(A[0][:], dum[:, 0:P], dum[:],
                              start=True, stop=True, skip_group_check=True)
            for _i in range(9):
                tensor.matmul(A[0][:, 0:P], dum[:, 0:P], dum[:, 0:P],
                              start=True, stop=True, skip_group_check=True)
            tensor.wait_ge(dsem_x, 16)   # xr kj0-1 blocks resident
            # all 16 exponent matmuls back-to-back (PE stays busy/ramped)
            for kj in range(KJ):
                if kj == 2:
                    tensor.wait_ge(dsem_x2, 16)   # remaining xr resident
                for ki in range(KIW):
                    mm = tensor.matmul(
                        A[kj][:, ki * P:(ki + 1) * P],
                        lxh[kj][:, ki * P:(ki + 1) * P],
                        rth[kj][:],
                        start=True,
                        stop=True,
                        skip_group_check=True,
                    )
                    if ki == KIW - 1:
                        mm.then_inc(psem, 1)
            tensor.wait_ge(dsem_y, 16)   # ydt resident
            for kj in range(KJ):
                tensor.wait_ge(asem, kj + 1)
                for ki in range(KIW):
                    mm = tensor.matmul(
                        pacc[kj][:, 0:CW],
                        E[kj][:, ki * P:(ki + 1) * P],
                        ydt[:, (kj * KIW + ki) * CW:(kj * KIW + ki + 1) * CW],
                        start=(ki == 0),
                        stop=(ki == KIW - 1),
                        skip_group_check=True,
                    )
                    if ki == KIW - 1:
                        mm.then_inc(gsem, 1)

        @block.scalar
        def _(scalar):
            # ydt DMA rides the scalar queue (sync is busy with xr);
            # issued before the warm-up so the transfer overlaps table load
            scalar.dma_start(out=ydt[:], in_=ydt_d[:]).then_inc(dsem_y, 16)
            # touch Exp before the pipeline needs it: loads the ACT table
            # while the input DMAs are still in flight
            scalar.activation(warm[:], nc.const_aps.tensor(0.0, (1, 1)), Exp)
            for kj in range(KJ):
                scalar.wait_ge(psem, kj + 1)
                scalar.activation(E[kj][:], A[kj][:], Exp).then_inc(asem, 1)
            # after the Exps, ACT picks up kj2's P*recip so the DVE epilogue
            # isn't serialized on one engine (kj3's stays on DVE: routing it
            # through ACT would put two sem hops on the final critical add)
            Copy = mybir.ActivationFunctionType.Copy
            scalar.wait_ge(rsem, 3)
            scalar.activation(
                prod[:, 2 * OUT_CH:3 * OUT_CH],
                pacc[2][:, 1:1 + OUT_CH],
                Copy,
                scale=recip4[:, 2:3],
            ).then_inc(tsem, 1)

        @block.vector
        def _(vector):
            # per-kj: recip = 1/(den+eps), prod = P*recip (PSUM->SBUF),
            # pipelined so the self-pipe waits are met when reached; then
            # one strided 3D add folds in the den*W0 columns for all kj
            def recip_op(kj):
                vector.wait_ge(gsem, kj + 1)
                vector.reciprocal(
                    recip4[:, kj:kj + 1], pacc[kj][:, 0:1]
                ).then_inc(rsem, 1)

            def mul_op(kj):
                vector.wait_ge(rsem, kj + 1)
                vector.tensor_scalar_mul(
                    prod[:, kj * OUT_CH:(kj + 1) * OUT_CH],
                    pacc[kj][:, 1:1 + OUT_CH],
                    recip4[:, kj:kj + 1],
                ).then_inc(qsem, 1)

            def add_op(kj, sem, val):
                vector.wait_ge(sem, val)
                vector.tensor_tensor(
                    o_all[:, kj * OUT_CH:(kj + 1) * OUT_CH],
                    pacc[kj][:, 1 + OUT_CH:CW],
                    prod[:, kj * OUT_CH:(kj + 1) * OUT_CH],
                    op=mybir.AluOpType.add,
                ).then_inc(vsem, 1)

            # DVE handles kj0/kj1/kj3's P*recip; ACT (free after its Exps)
            # handles kj2's, so the tails run in parallel and kj3's final
            # add never crosses engines
            recip_op(0); mul_op(0)
            recip_op(1); mul_op(1)
            add_op(0, qsem, 1)
            recip_op(2)
            add_op(1, qsem, 2)
            recip_op(3); mul_op(3)
            add_op(2, tsem, 1)
            add_op(3, qsem, 3)

    return nc


def _build_fp16_raw(widths):
    """Dense fallback: raw-Bass fp16 pipeline over all 1024 context points
    (used when the banded window bound is not accurate enough)."""
    import concourse.bass as bass
    from concourse import mybir

    f32 = mybir.dt.float32
    f16 = mybir.dt.float16
    G = len(widths)
    wtot = sum(widths)
    offs = np.cumsum([0] + list(widths))
    npair = NPAIR * G
    Exp = mybir.ActivationFunctionType.Exp

    nc = bass.Bass("TRN2", target_bir_lowering=False, debug=False)

    # xr = per-group [lxh | rth] blocks along the free dim (groups must sit at
    # partition 0 for PE); one DMA feeds the first matmul. wbb = [wb0 | bt].
    BLK = N_IN + O_CORE
    xr_d = nc.dram_tensor("xr", [KEXP, G * BLK], f16, kind="ExternalInput")
    ydt_d = nc.dram_tensor("ydt", [P, KI * wtot], f16, kind="ExternalInput")
    wbb_d = nc.dram_tensor("wbb", [P, 2 * OUT_CH], f32, kind="ExternalInput")
    out_d = nc.dram_tensor("out", [O_CORE, OUT_CH], f32, kind="ExternalOutput")

    xr = nc.alloc_sbuf_tensor("xr_sb", [KEXP, G * BLK], f16).ap()
    lxh = [xr[:, g * BLK:g * BLK + N_IN] for g in range(G)]
    rth = [xr[:, g * BLK + N_IN:(g + 1) * BLK] for g in range(G)]
    ydt = nc.alloc_sbuf_tensor("ydt_sb", [P, KI * wtot], f16).ap()
    wbb = nc.alloc_sbuf_tensor("wbb_sb", [P, 2 * OUT_CH], f32).ap()
    wb0 = wbb[:, :OUT_CH]
    bt = wbb[:, OUT_CH:]
    warm = nc.alloc_sbuf_tensor("warm_sb", [1, 1], f32).ap()
    E = [nc.alloc_sbuf_tensor(f"e_sb{q}", [P, 2 * O_CORE], f16).ap()
         for q in range(npair)]
    denp = [nc.alloc_sbuf_tensor(f"denp_sb{kj}", [P, 1], f32).ap()
            for kj in range(KJ)]
    recip = [nc.alloc_sbuf_tensor(f"recip_sb{kj}", [P, 1], f32).ap()
             for kj in range(KJ)]
    t1 = [nc.alloc_sbuf_tensor(f"t1_sb{kj}", [P, OUT_CH], f32).ap()
          for kj in range(KJ)]
    o_all = nc.alloc_sbuf_tensor("o_all_sb", [P, KJ * OUT_CH], f32).ap()
    o_sb = [o_all[:, kj * OUT_CH:(kj + 1) * OUT_CH] for kj in range(KJ)]
    A = [nc.alloc_psum_tensor(f"a_ps{i}", [P, 2 * O_CORE], f32).ap()
         for i in range(2)]
    pacc = [nc.alloc_psum_tensor(f"pacc{kj}", [P, 1 + OUT_CH], f32).ap()
            for kj in range(KJ)]

    with (
        nc.Block() as block,
        nc.semaphore("dsem_s") as dsem_s,   # sync-queue input DMAs
        nc.semaphore("dsem_g") as dsem_g,   # gpsimd-queue input DMAs
        nc.semaphore("psem") as psem,       # PE exponent matmuls done
        nc.semaphore("asem") as asem,       # ACT exp pairs done
        nc.semaphore("gsem") as gsem,       # PE agg per-kj done
        nc.semaphore("vsem") as vsem,       # DVE epilogue per-kj done
        nc.semaphore("vv") as vv,           # DVE same-engine pipeline sync
        nc.semaphore("osem") as osem,       # out DMAs done
    ):
        @block.sync
        def _(sync):
            sync.dma_start(out=xr[:], in_=xr_d[:]).then_inc(dsem_s, 16)
            for kj in range(KJ):
                sync.wait_ge(vsem, kj + 1)
                sync.dma_start(
                    out=out_d[kj * P:(kj + 1) * P, :], in_=o_sb[kj][:]
                ).then_inc(osem, 16)

        @block.gpsimd
        def _(gpsimd):
            gpsimd.dma_start(out=ydt[:], in_=ydt_d[:]).then_inc(dsem_g, 16)
            gpsimd.dma_start(out=wbb[:], in_=wbb_d[:]).then_inc(dsem_g, 16)

        @block.tensor
        def _(tensor):
            tensor.wait_ge(dsem_s, 16)
            for q in range(npair):
                g, p = divmod(q, NPAIR)
                if q >= 2:
                    tensor.wait_ge(asem, q - 1)  # A buffer q%2 free again
                for half in range(2):
                    ki = 2 * p + half
                    tensor.matmul(
                        A[q % 2][:, half * O_CORE:(half + 1) * O_CORE],
                        lxh[g][:, ki * P:(ki + 1) * P],
                        rth[g][:],
                        start=True,
                        stop=True,
                    ).then_inc(psem, 1)
            tensor.wait_ge(dsem_g, 32)  # ydt (all gpsimd-queue DMAs)
            n_mm = G * KI
            for kj in range(KJ):
                for g in range(G):
                    w = widths[g]
                    coff = 0 if g == 0 else 1
                    for ki in range(KI):
                        idx = g * KI + ki
                        q = g * NPAIR + ki // 2
                        if kj == 0 and ki % 2 == 0:
                            tensor.wait_ge(asem, q + 1)
                        rhs = ydt[:, KI * offs[g] + ki * w:
                                  KI * offs[g] + (ki + 1) * w]
                        lhs = E[q][:, (ki % 2) * O_CORE + kj * P:
                                   (ki % 2) * O_CORE + (kj + 1) * P]
                        mm = tensor.matmul(
                            pacc[kj][:, coff:coff + w],
                            lhs,
                            rhs,
                            start=(idx == 0),
                            stop=(idx == n_mm - 1),
                            skip_group_check=(G > 1),
                        )
                        if idx == n_mm - 1:
                            mm.then_inc(gsem, 1)

        @block.scalar
        def _(scalar):
            scalar.activation(warm[:], nc.const_aps.tensor(0.0, (1, 1)), Exp)
            for q in range(npair):
                scalar.wait_ge(psem, 2 * (q + 1))
                scalar.activation(E[q][:], A[q % 2][:], Exp).then_inc(asem, 1)

        @block.vector
        def _(vector):
            vector.wait_ge(dsem_g, 32)  # wbb resident
            for kj in range(KJ):
                vector.wait_ge(gsem, kj + 1)
                vector.tensor_scalar_add(
                    denp[kj][:], pacc[kj][:, 0:1], EPS
                ).then_inc(vv, 1)
                vector.wait_ge(vv, 3 * kj + 1)  # denp through the pipe
                vector.scalar_tensor_tensor(
                    t1[kj][:], wb0[:], denp[kj][:], bt[:],
                    op0=mybir.AluOpType.mult, op1=mybir.AluOpType.add,
                ).then_inc(vv, 1)
                vector.reciprocal(recip[kj][:], denp[kj][:]).then_inc(vv, 1)
                vector.wait_ge(vv, 3 * kj + 3)  # t1 + recip through the pipe
                vector.scalar_tensor_tensor(
                    o_sb[kj][:], pacc[kj][:, 1:1 + OUT_CH], recip[kj][:], t1[kj][:],
                    op0=mybir.AluOpType.mult, op1=mybir.AluOpType.add,
                ).then_inc(vsem, 1)

    return nc


def _build_fp32(widths):
    """Fallback: fp32 rank-3 exponent matmul + fp32 aggregation (slower,
    used only when fp16 split values would overflow)."""
    import concourse.bacc as bacc
    import concourse.tile as tile
    from concourse import mybir

    f32 = mybir.dt.float32
    G = len(widths)
    wtot = sum(widths)
    offs = np.cumsum([0] + list(widths))

    nc = bacc.Bacc("TRN2", target_bir_lowering=False, debug=False)

    lx_d = nc.dram_tensor("lx", [3, N_IN], f32, kind="ExternalInput")
    rt_d = nc.dram_tensor("rt", [3 * G, O_CORE], f32, kind="ExternalInput")
    ydt_d = nc.dram_tensor("ydt", [P, KI * wtot], f32, kind="ExternalInput")
    wb0_d = nc.dram_tensor("wb0", [P, OUT_CH], f32, kind="ExternalInput")
    bt_d = nc.dram_tensor("bt", [P, OUT_CH], f32, kind="ExternalInput")
    out_d = nc.dram_tensor("out", [O_CORE, OUT_CH], f32, kind="ExternalOutput")

    with tile.TileContext(nc) as tc:
        with (
            tc.tile_pool(name="const", bufs=1) as cpool,
            tc.tile_pool(name="epool", bufs=1) as epool,
            tc.tile_pool(name="small", bufs=2) as spool,
            tc.tile_pool(name="outp", bufs=2) as opool,
            tc.tile_pool(name="apsum", bufs=3, space="PSUM") as apsum,
            tc.tile_pool(name="ppsum", bufs=1, space="PSUM") as ppsum,
        ):
            lx = cpool.tile([3, N_IN], f32, tag="lx")
            nc.sync.dma_start(lx[:], lx_d[:])
            rt = cpool.tile([3 * G, O_CORE], f32, tag="rt")
            nc.scalar.dma_start(rt[:], rt_d[:])
            ydt = cpool.tile([P, KI * wtot], f32, tag="ydt")
            nc.gpsimd.dma_start(ydt[:], ydt_d[:])
            wb0 = cpool.tile([P, OUT_CH], f32, tag="wb0")
            nc.gpsimd.dma_start(wb0[:], wb0_d[:])
            bt = cpool.tile([P, OUT_CH], f32, tag="bt")
            nc.gpsimd.dma_start(bt[:], bt_d[:])

            E = {}
            for g in range(G):
                for ki in range(KI):
                    a_ps = apsum.tile([P, O_CORE], f32, tag="A", name="a_ps")
                    nc.tensor.matmul(
                        a_ps[:],
                        lx[:, ki * P:(ki + 1) * P],
                        rt[3 * g:3 * g + 3, :],
                        start=True,
                        stop=True,
                    )
                    e = epool.tile([P, O_CORE], f32, tag=f"E{g}_{ki}", name="e")
                    nc.scalar.activation(
                        e[:], a_ps[:], mybir.ActivationFunctionType.Exp
                    )
                    E[(g, ki)] = e

            pacc = [
                ppsum.tile([P, 1 + OUT_CH], f32, tag=f"P{kj}", name=f"pacc{kj}")
                for kj in range(KJ)
            ]
            n_mm = G * KI
            for kj in range(KJ):
                for g in range(G):
                    w = widths[g]
                    coff = 0 if g == 0 else 1
                    for ki in range(KI):
                        idx = g * KI + ki
                        rhs = ydt[:, KI * offs[g] + ki * w: KI * offs[g] + (ki + 1) * w]
                        nc.tensor.matmul(
                            pacc[kj][:, coff:coff + w],
                            E[(g, ki)][:, kj * P:(kj + 1) * P],
                            rhs,
                            start=(idx == 0),
                            stop=(idx == n_mm - 1),
                            skip_group_check=(G > 1),
                        )

                denp = spool.tile([P, 1], f32, tag="denp", name="denp")
                nc.vector.tensor_scalar_add(denp[:], pacc[kj][:, 0:1], EPS)
                recip = spool.tile([P, 1], f32, tag="recip", name="recip")
                nc.vector.reciprocal(recip[:], denp[:])
                t1 = spool.tile([P, OUT_CH], f32, tag="t1", name="t1")
                nc.vector.scalar_tensor_tensor(
                    t1[:], wb0[:], denp[:], bt[:],
                    op0=mybir.AluOpType.mult, op1=mybir.AluOpType.add,
                )
                o_sb = opool.tile([P, OUT_CH], f32, tag="osb", name="o_sb")
                nc.vector.scalar_tensor_tensor(
                    o_sb[:], pacc[kj][:, 1:1 + OUT_CH], recip[:], t1[:],
                    op0=mybir.AluOpType.mult, op1=mybir.AluOpType.add,
                )
                nc.sync.dma_start(out_d[kj * P:(kj + 1) * P, :], o_sb[:])

    nc.compile()
    return nc


def _split2_f16(v):
    """2-way fp16 split: v ~= h1 + h2 with each half exactly fp16."""
    v = v.astype(np.float32)
    h1 = v.astype(np.float16)
    h2 = (v - h1.astype(np.float32)).astype(np.float16)
    return h1, h2


def _prepare_banded(context_x, context_y, t, sigma, W, b):
    """Host prep for the banded kernel. Returns (in_maps, gathers) or None
    when the banded path does not apply (multi-sigma, fp16 overflow risk,
    or a window accuracy bound too loose)."""
    sigma = np.asarray(sigma, dtype=np.float32)
    if not np.all(sigma == sigma[0]):
        return None
    a = 0.5 / np.exp(2.0 * np.float64(sigma[0]))
    if not np.isfinite(a):
        return None
    r = np.sqrt(2.0 * a)

    cx = np.asarray(context_x, dtype=np.float64)[:, :, 0]
    tt = np.asarray(t, dtype=np.float64)[:, :, 0]
    xmax = max(float(np.abs(cx).max()), float(np.abs(tt).max()), 1.0)
    if not (a * xmax * xmax < 3e4):
        return None

    W64 = np.asarray(W, dtype=np.float64)
    b64 = np.asarray(b, dtype=np.float64)
    WR = W_WIN - 1   # real context points per window; slot 511 is the eps row

    in_maps, gathers = [], []
    for core in range(N_CORES):
        bidx, half = core // 2, core % 2
        xo = np.argsort(cx[bidx], kind="stable")
        xs = cx[bidx][xo]
        to = np.argsort(tt[bidx], kind="stable")
        ts = tt[bidx][to]
        # folded context values in sorted-x order: [1 | y@W' + b | W0]
        u = np.asarray(context_y[bidx], np.float64) @ W64[1:] + b64
        u_s = u[xo]

        xr = np.empty((KEXP, KJ * BBLK), dtype=np.float16)
        ydt = np.empty((P, KJ * KIW * CW), dtype=np.float16)
        tidx = np.empty(O_CORE, dtype=np.int64)
        cand = np.arange(0, N_IN - WR + 1)
        for kj in range(KJ):
            lo = half * O_CORE + kj * P
            tc = ts[lo:lo + P]
            tidx[kj * P:(kj + 1) * P] = to[lo:lo + P]
            # window start maximizing the smaller margin
            m = np.minimum(tc.min() - xs[cand], xs[cand + WR - 1] - tc.max())
            s = int(cand[np.argmax(m)])
            xw = xs[s:s + WR]
            # accuracy bound: dropped tail mass on the density channel
            drop = np.concatenate([xs[:s], xs[s + WR:]])
            if drop.size:
                d = np.maximum(0.0, np.maximum(tc.min() - drop,
                                               drop - tc.max()))
                if float(np.exp(-a * d * d).sum()) > 5e-4:
                    return None
            s1, s2 = _split2_f16(r * xw)
            u1, u2 = _split2_f16(r * tc)
            q1, q2 = _split2_f16(0.5 * (r * xw) ** 2)
            w1, w2 = _split2_f16(0.5 * (r * tc) ** 2)
            one_i = np.ones(WR, np.float16)
            neg1 = np.full(P, -1.0, np.float16)
            lblk = np.zeros((KEXP, W_WIN), dtype=np.float16)
            lblk[:, :WR] = np.stack([s1, s1, s2, s2, q1, q2, one_i, one_i])
            # eps row (slot 511): exponent = -9.21034 -> E = 1e-4, and the
            # density ydt value is 1e-4, so E*ydt = 1e-8 = eps with both
            # factors fp16-normal (1e-8 itself would underflow fp16)
            lblk[4, WR] = np.float16(9.21034)
            xr[:, kj * BBLK:kj * BBLK + W_WIN] = lblk
            xr[:, kj * BBLK + W_WIN:(kj + 1) * BBLK] = np.stack(
                [u1, u2, u1, u2, neg1, neg1, -w1, -w2]
            )
            blk = np.zeros((W_WIN, CW))
            blk[:WR, 0] = 1.0
            blk[:WR, 1:1 + OUT_CH] = u_s[s:s + WR]
            blk[:WR, 1 + OUT_CH:] = W64[0][None, :]
            blk[WR, 0] = 1e-4   # with E = 1e-4: pacc[:,0] = den + 1e-8
            ydt[:, kj * KIW * CW:(kj + 1) * KIW * CW] = (
                blk.reshape(KIW, P, CW).transpose(1, 0, 2).reshape(P, KIW * CW)
            ).astype(np.float16)

        in_maps.append({"xr": xr, "ydt": ydt})
        gathers.append((bidx, tidx))
    return in_maps, gathers


def _prepare_inputs(context_x, context_y, t, sigma, W, b):
    """Dense-path host prep: group channels by sigma, fold W, build
    per-core inputs."""
    sigma = np.asarray(sigma, dtype=np.float32)
    W64 = np.asarray(W, dtype=np.float64)
    b64 = np.asarray(b, dtype=np.float64)

    uniq = []
    for c in range(IN_CH):
        if sigma[c] not in uniq:
            uniq.append(sigma[c])
    uniq.sort(key=lambda s: (s != sigma[0]))  # channel-0 group first
    groups = [[c for c in range(IN_CH) if sigma[c] == s] for s in uniq]
    alphas = [0.5 / np.exp(2.0 * np.float64(s)) for s in uniq]
    widths = tuple((1 + OUT_CH) if 0 in g else OUT_CH for g in groups)
    G = len(groups)

    xmax = max(
        float(np.abs(np.asarray(context_x)).max()),
        float(np.abs(np.asarray(t)).max()),
        1.0,
    )
    fp16_ok = all(a * xmax * xmax < 3e4 and np.isfinite(a) for a in alphas)

    in_maps = []
    for core in range(N_CORES):
        bidx, half = core // 2, core % 2
        x = np.asarray(context_x[bidx, :, 0], dtype=np.float64)
        th = np.asarray(t[bidx, half * O_CORE:(half + 1) * O_CORE, 0],
                        dtype=np.float64)
        y = np.asarray(context_y[bidx], dtype=np.float64)

        m = {}
        if fp16_ok:
            BLK = N_IN + O_CORE
            xr = np.empty((KEXP, G * BLK), dtype=np.float16)
            for g, a in enumerate(alphas):
                r = np.sqrt(2.0 * a)
                s1, s2 = _split2_f16(r * x)
                u1, u2 = _split2_f16(r * th)
                q1, q2 = _split2_f16(0.5 * (r * x) ** 2)
                w1, w2 = _split2_f16(0.5 * (r * th) ** 2)
                one_i = np.ones(N_IN, np.float16)
                neg1 = np.full(O_CORE, -1.0, np.float16)
                xr[:, g * BLK:g * BLK + N_IN] = np.stack(
                    [s1, s1, s2, s2, q1, q2, one_i, one_i]
                )
                xr[:, g * BLK + N_IN:(g + 1) * BLK] = np.stack(
                    [u1, u2, u1, u2, neg1, neg1, -w1, -w2]
                )
            m["xr"] = xr
        else:
            lx = np.stack([x, x * x, np.ones_like(x)]).astype(np.float32)
            rt = np.empty((3 * G, O_CORE), dtype=np.float32)
            for g, a in enumerate(alphas):
                rt[3 * g + 0] = 2.0 * a * th
                rt[3 * g + 1] = -a
                rt[3 * g + 2] = -a * th * th
            m["lx"], m["rt"] = lx, rt

        blocks = []
        for g, chans in enumerate(groups):
            w = widths[g]
            rhs = np.zeros((N_IN, w), dtype=np.float64)
            coff = 0
            if 0 in chans:
                rhs[:, 0] = 1.0
                coff = 1
            conv_ch = [c for c in chans if c > 0]
            if conv_ch:
                rhs[:, coff:] = y[:, [c - 1 for c in conv_ch]] @ W64[conv_ch, :]
            blocks.append(
                rhs.reshape(KI, P, w).transpose(1, 0, 2).reshape(P, KI * w)
            )
        ydt = np.concatenate(blocks, axis=1)
        m["ydt"] = ydt.astype(np.float16 if fp16_ok else np.float32)
        wb0 = np.tile(W64[0].astype(np.float32), (P, 1))
        bt = np.tile(b64.astype(np.float32), (P, 1))
        if fp16_ok:
            m["wbb"] = np.concatenate([wb0, bt], axis=1)
        else:
            m["wb0"], m["bt"] = wb0, bt
        in_maps.append(m)
    return widths, fp16_ok, in_maps


def _run(inputs: dict, trace: bool = False):
    """Compile (cached), run on 8 cores, gather. Returns (output, results)."""
    from concourse.bass_utils import run_bass_kernel_spmd

    banded = _prepare_banded(
        inputs["context_x"], inputs["context_y"], inputs["t"],
        inputs["sigma"], inputs["W"], inputs["b"],
    )
    out = np.empty((B, N_OUT, OUT_CH), dtype=np.float32)
    if banded is not None:
        in_maps, gathers = banded
        if "banded" not in _BASS_CACHE:
            _BASS_CACHE["banded"] = _build_banded()
        nc = _BASS_CACHE["banded"]
        res = run_bass_kernel_spmd(nc, in_maps, list(range(N_CORES)),
                                   trace=trace)
        for core in range(N_CORES):
            bidx, tidx = gathers[core]
            r = res.results[core]["out"]  # [128, KJ*16]
            out[bidx, tidx, :] = (
                r.reshape(P, KJ, OUT_CH).transpose(1, 0, 2).reshape(O_CORE,
                                                                    OUT_CH)
            )
        return out, res

    widths, fp16_ok, in_maps = _prepare_inputs(
        inputs["context_x"], inputs["context_y"], inputs["t"],
        inputs["sigma"], inputs["W"], inputs["b"],
    )
    key = (widths, fp16_ok)
    if key not in _BASS_CACHE:
        _BASS_CACHE[key] = (_build_fp16_raw if fp16_ok else _build_fp32)(widths)
    nc = _BASS_CACHE[key]

    res = run_bass_kernel_spmd(nc, in_maps, list(range(N_CORES)), trace=trace)

    for core in range(N_CORES):
        bidx, half = core // 2, core % 2
        out[bidx, half * O_CORE:(half + 1) * O_CORE, :] = res.results[core]["out"]
    return out, res


def kernel(**inputs) -> np.ndarray:
    out, _ = _run(inputs, trace=False)
    return out
